# revision 27
# baseline (speedup 1.0000x reference)
"""Trainium2 Bass kernel for nn_Decoder (GRU decoder over padded sequences).

Computation (per sample):
  emb = message[:, :T-1] @ W_emb.T + b_emb            (folded into W_c on host)
  xs  = [init_emb, emb]                                (step 0 folded into h1 const)
  GRU over T steps, gather h at lengths-1              (freeze trick: z := 1 past len)
  out = sigmoid(elu(h @ W1.T + b1) @ W2.T + b2)

Sharding: batch data-parallel over 8 cores, host-side length-sort (stratified
round-robin across cores) so each chunk only runs to its max length.

Device layout: features on partitions, batch on the free dim. The GRU state h
(and the weights it multiplies) live at partition base 64 so that the z-gate
slice of the sigmoid output tile (partitions 64..127) is base-aligned with the
h-update operands — every elementwise op's SBUF operands share a start
partition, which the ISA requires. Chunks are processed in pairs (1024
samples) with the h-update ops fused at FD=1024, and pairs are interleaved
PIPE at a time so the serial per-step dependency chain overlaps.
"""

import sys

sys.path.insert(0, "/opt/trn_rl_repo")

import numpy as np
import ml_dtypes

import concourse.bacc as bacc
import concourse.mybir as mybir
import concourse.tile as tile
from concourse.bass_utils import run_bass_kernel_spmd

B, T, V, E, H, FC, OUT = 65536, 30, 21, 32, 64, 256, 784
NCORES = 8
BC = B // NCORES          # 8192 samples per core
CH = 512                  # matmul free dim (one PSUM bank)
CH2 = 2 * CH              # pair width
NP = BC // CH2            # 8 pairs per core
FREEZE = 40.0             # z-gate preactivation offset for finished samples
KX = V + 1                # 21 msg rows + 1 freeze-flag row
OTILES = (OUT + 127) // 128  # 7 output row tiles
PIPE = 8                  # pairs in flight (all interleaved)
XB = 2                    # GRU steps per streamed X block

USE_BF16 = True
GP_HP = False             # final h-update add on GpSimd (else VectorE)
HN_COPY = True            # stage hn PSUM->SBUF via ScalarE so t1 runs at 2x
TRACE = False             # set by test harness for profiling
LAST_RESULT = None        # BassKernelResults stash for the harness

_f32 = mybir.dt.float32
_bf16 = mybir.dt.bfloat16


def _sigmoid(x):
    return 1.0 / (1.0 + np.exp(-x))


def _build_nc(pair_steps, dt):
    """Build the SPMD kernel. pair_steps[p] = GRU steps (beyond the constant
    step 0) for pair p — identical on every core."""
    AF = mybir.ActivationFunctionType
    OP = mybir.AluOpType
    nc = bacc.Bacc("TRN2", target_bir_lowering=False, debug=False)

    X = nc.dram_tensor("X", [T - 1, KX, BC], dt, kind="ExternalInput")
    WX = nc.dram_tensor("WX", [KX, 3 * H], dt, kind="ExternalInput")
    WH = nc.dram_tensor("WH", [H, 3 * H], dt, kind="ExternalInput")
    W1T = nc.dram_tensor("W1T", [H, FC], dt, kind="ExternalInput")
    W2T = nc.dram_tensor("W2T", [FC, OUT], dt, kind="ExternalInput")
    BRZ = nc.dram_tensor("BRZ", [2 * H, 1], _f32, kind="ExternalInput")
    B1 = nc.dram_tensor("B1", [128, FC // 128], _f32, kind="ExternalInput")
    B2 = nc.dram_tensor("B2", [128, OTILES], _f32, kind="ExternalInput")
    H1 = nc.dram_tensor("H1", [H, 1], _f32, kind="ExternalInput")
    BHB = nc.dram_tensor("BHB", [1, H], dt, kind="ExternalInput")
    BCB = nc.dram_tensor("BCB", [1, H], dt, kind="ExternalInput")
    OT = nc.dram_tensor("OT", [OUT, BC], _f32, kind="ExternalOutput")

    ew = nc.gpsimd if GP_HP else nc.vector

    with tile.TileContext(nc) as tc:
        with (
            tc.tile_pool(name="weights", bufs=1) as wp,
            tc.tile_pool(name="xin", bufs=9) as xp,
            tc.tile_pool(name="hstate", bufs=NP) as hp,
            tc.tile_pool(name="hfinal", bufs=1) as hf,
            tc.tile_pool(name="gates", bufs=3) as gp,
            tc.tile_pool(name="head", bufs=3) as fp,
            tc.tile_pool(name="frhs", bufs=1) as frp,
            tc.tile_pool(name="outs", bufs=3) as op_,
            tc.tile_pool(name="psA", bufs=2, space="PSUM") as psA,
            tc.tile_pool(name="psB", bufs=2, space="PSUM") as psB,
        ):
            # --- load weights/biases once; h-side weights live at base 64 ---
            wx = wp.tile([128, 3 * H], dt)
            nc.vector.memset(wx[:], 0.0)
            nc.sync.dma_start(out=wx[0:KX, :], in_=WX[:])
            # ones-row bias folding: x row KX is 1.0, h row 0 is 1.0; the
            # matching weight rows carry b_cn / b_hn into the n-path PSUM.
            nc.sync.dma_start(out=wx[32:33, 2 * H :], in_=BCB[:])
            whF = wp.tile([128, 3 * H], dt)
            nc.vector.memset(whF[0:H, :], 0.0)
            nc.sync.dma_start(out=whF[H:128, :], in_=WH[:])
            nc.sync.dma_start(out=whF[0:1, 2 * H :], in_=BHB[:])
            w1F = wp.tile([128, FC], dt)
            nc.vector.memset(w1F[0:H, :], 0.0)
            nc.sync.dma_start(out=w1F[H:128, :], in_=W1T[:])
            w2a = wp.tile([128, OUT], dt)
            nc.sync.dma_start(out=w2a[:], in_=W2T[0:128, :])
            w2b = wp.tile([128, OUT], dt)
            nc.sync.dma_start(out=w2b[:], in_=W2T[128:256, :])
            brz = wp.tile([2 * H, 1], _f32)
            nc.sync.dma_start(out=brz[:], in_=BRZ[:])
            b1s = wp.tile([128, FC // 128], _f32)
            nc.sync.dma_start(out=b1s[:], in_=B1[:])
            b2s = wp.tile([128, OTILES], _f32)
            nc.sync.dma_start(out=b2s[:], in_=B2[:])
            h1F = wp.tile([128, 1], _f32)
            nc.sync.dma_start(out=h1F[H:128, :], in_=H1[:])

            hfin = []

            # PE warm-up: ~5us of dense junk matmuls so HAM un-throttles the
            # clock gate (4/8 -> 8/8) before the GRU loops start.
            warm = psA.tile([128, CH2], _f32, tag="ps")
            for _ in range(30):
                nc.tensor.matmul(
                    warm[:, 0 : 3 * H], wx[:, 0:128], wx[:], start=True, stop=True
                )

            for _xi in range(9):
                xz = xp.tile([128, XB, CH2], dt, tag="xt")
                nc.vector.memset(xz[:], 0.0)
                nc.vector.memset(xz[32:33, :, :], 1.0)

            def start_pair(p):
                """Allocate tiles + init h for pair p; returns state dict."""
                ha = hp.tile([128, CH2], dt, tag="hpi")
                hb = hp.tile([128, CH2], dt, tag="hpo")
                hfp = hf.tile([128, CH2], dt, tag=f"hf{p}")
                hfin.append(hfp)
                nc.vector.memset(ha[:], 0.0)
                nc.vector.memset(ha[0:1, :], 1.0)
                nc.vector.memset(hb[0:H, :], 0.0)
                nc.vector.memset(hb[0:1, :], 1.0)
                nc.vector.memset(hfp[0:H, :], 0.0)
                nc.vector.memset(hfp[0:1, :], 1.0)
                nc.vector.tensor_scalar_add(ha[H:128, :], ha[H:128, :], h1F[H:128, :])
                if pair_steps[p] == 0:
                    nc.vector.memset(hfp[H:128, :], 0.0)
                    nc.vector.tensor_scalar_add(
                        hfp[H:128, :], hfp[H:128, :], h1F[H:128, :]
                    )
                return {"xt": None, "cur": ha, "nxt": hb, "hf": hfp, "p": p}

            def emit_mms(bite, s):
                """Gate matmuls for up to 2 pairs at step s, grouped by weight
                matrix so consecutive MMs share LDWEIGHTS and pipeline."""
                for st in bite:
                    nsteps = pair_steps[st["p"]]
                    if (s - 1) % XB == 0:
                        nb = min(XB, nsteps - (s - 1))
                        xtn = xp.tile([128, XB, CH2], dt, tag="xt")
                        st["xt"] = xtn
                        ps = slice(st["p"] * CH2, (st["p"] + 1) * CH2)
                        nc.sync.dma_start(
                            out=st["xt"][0:KX, 0:nb, :],
                            in_=X[s - 1 : s - 1 + nb, :, ps].rearrange("t k b -> k t b"),
                        )
                    # 2-bank PSUM tiles: both chunks side by side -> FD=1024 ops.
                    # pn layout: hn on partitions 0..63, xn on partitions 64..127.
                    prz2 = psA.tile([128, CH2], _f32, tag="ps")
                    pn2 = psB.tile([128, CH2], _f32, tag="pn")
                    st["prz2"], st["pn2"] = prz2, pn2

                def each(fn):
                    for st in bite:
                        j = (s - 1) % XB
                        for g in (0, 1):
                            gs = slice(g * CH, (g + 1) * CH)
                            fn(st, st["xt"][:, j, gs], st["cur"][:, gs], gs)

                each(lambda st, xs_, cur_g, gs: nc.tensor.matmul(
                    st["prz2"][:, gs], wx[:, 0:128], xs_, start=True, stop=False))
                each(lambda st, xs_, cur_g, gs: nc.tensor.matmul(
                    st["prz2"][:, gs], whF[:, 0:128], cur_g, start=False, stop=True))
                each(lambda st, xs_, cur_g, gs: nc.tensor.matmul(
                    st["pn2"][0:H, gs], whF[:, 128:192], cur_g, start=True, stop=True))
                each(lambda st, xs_, cur_g, gs: nc.tensor.matmul(
                    st["pn2"][H:128, gs], wx[:, 128:192], xs_, start=True, stop=True))

            def emit_step(st, s):
                """Elementwise GRU step for both chunks of a pair."""
                cur = st["cur"]
                nsteps = pair_steps[st["p"]]
                prz2, pn2 = st["prz2"], st["pn2"]
                rzF = gp.tile([128, CH2], dt, tag="rz")
                nc.scalar.activation(rzF[:], prz2[:], AF.Sigmoid, bias=brz[:])
                # biases b_hn / b_cn already folded into pn2 via ones-rows
                t1 = gp.tile([H, CH2], dt, tag="t1")
                nc.vector.tensor_mul(t1[:], pn2[0:H, :], rzF[0:H, :])
                t2F = gp.tile([H, CH2], dt, tag="t2")
                nc.vector.tensor_add(t2F[:], pn2[H:128, :], t1[:])
                # n = tanh(t2);  h' = n + z*(h - n)
                ntF = gp.tile([128, CH2], dt, tag="nt")
                nc.scalar.activation(ntF[H:128, :], t2F[:], AF.Tanh)
                uF = gp.tile([128, CH2], dt, tag="u")
                nc.vector.tensor_sub(uF[H:128, :], cur[H:128, :], ntF[H:128, :])
                vF = gp.tile([128, CH2], dt, tag="v")
                nc.vector.tensor_mul(vF[H:128, :], rzF[H:128, :], uF[H:128, :])
                dst = st["hf"] if s == nsteps else st["nxt"]
                ew.tensor_add(dst[H:128, :], ntF[H:128, :], vF[H:128, :])
                st["cur"], st["nxt"] = dst, st["cur"]

            # --- GRU loops: all pairs interleaved so short pairs' tails
            # overlap long pairs' matmul phases ---
            grp = [start_pair(p) for p in range(NP)]
            maxs = max(pair_steps[st["p"]] for st in grp)
            for s in range(1, maxs + 1):
                active = [st for st in grp if s <= pair_steps[st["p"]]]
                for i in range(0, len(active), 2):
                    bite = active[i : i + 2]
                    emit_mms(bite, s)
                    for st in bite:
                        emit_step(st, s)

            # --- MLP heads, two barrier-separated phases so ACT runs all its
            # Exp calls together (one table swap) then all Sigmoids ---
            tc.no_sync_barrier()
            f2rhs = {}
            for p in range(NP):
                for g in (0, 1):
                    gs = slice(g * CH, (g + 1) * CH)
                    cur_g = hfin[p][:, gs]
                    for j in range(FC // 128):
                        pfF = psA.tile([128, CH2], _f32, tag="ps")
                        pf = pfF[:, 0:CH]
                        nc.tensor.matmul(
                            pf, w1F[:, j * 128 : (j + 1) * 128], cur_g,
                            start=True, stop=True,
                        )
                        # elu(x+b1) + 1 == (x+b1 - m) + exp(m),  m = min(x+b1, 0)
                        m = fp.tile([128, CH], dt, tag="m")
                        nc.vector.tensor_scalar(
                            m[:], pf, b1s[:, j : j + 1], 0.0, op0=OP.add, op1=OP.min
                        )
                        e = fp.tile([128, CH], dt, tag="e")
                        nc.scalar.activation(e[:], m[:], AF.Exp)
                        pp = fp.tile([128, CH], dt, tag="pp")
                        nc.vector.scalar_tensor_tensor(
                            pp[:], pf, b1s[:, j : j + 1], m[:],
                            op0=OP.add, op1=OP.subtract,
                        )
                        fr = frp.tile([128, CH], dt, tag=f"fr{p}{g}{j}")
                        nc.vector.tensor_add(fr[:], pp[:], e[:])
                        f2rhs[(p, g, j)] = fr

            tc.no_sync_barrier()
            for p in range(NP):
                for g in (0, 1):
                    for mt in range(OTILES):
                        mw = min(128, OUT - mt * 128)
                        poF = psB.tile([128, CH2], _f32, tag="pn")
                        po = poF[0:mw, 0:CH]
                        nc.tensor.matmul(
                            po, w2a[:, mt * 128 : mt * 128 + mw],
                            f2rhs[(p, g, 0)][:], start=True, stop=False,
                        )
                        nc.tensor.matmul(
                            po, w2b[:, mt * 128 : mt * 128 + mw],
                            f2rhs[(p, g, 1)][:], start=False, stop=True,
                        )
                        ot = op_.tile([mw, CH], _f32, tag="ot")
                        nc.scalar.activation(
                            ot[:], po, AF.Sigmoid, bias=b2s[0:mw, mt : mt + 1]
                        )
                        nc.sync.dma_start(
                            out=OT[mt * 128 : mt * 128 + mw, p * CH2 + g * CH :
                                   p * CH2 + (g + 1) * CH],
                            in_=ot[:],
                        )

    nc.compile()
    return nc


def kernel(message, lengths, init_emb, W_emb, b_emb, W_ih, W_hh, b_ih, b_hh,
           W1, b1, W2, b2):
    global LAST_RESULT
    message = np.asarray(message, dtype=np.float32)
    lengths = np.asarray(lengths).astype(np.int64)
    f8 = np.float64
    np_dt = ml_dtypes.bfloat16 if USE_BF16 else np.float32
    dt = _bf16 if USE_BF16 else _f32

    # --- fold embedding into input weights;  step 0 is a constant ---
    W_c = W_ih.astype(f8) @ W_emb.astype(f8)                # [3H, V]
    b_c = W_ih.astype(f8) @ b_emb.astype(f8) + b_ih         # [3H]
    gx0 = W_ih.astype(f8) @ init_emb.astype(f8) + b_ih
    gh0 = b_hh.astype(f8)
    r0 = _sigmoid(gx0[:H] + gh0[:H])
    z0 = _sigmoid(gx0[H : 2 * H] + gh0[H : 2 * H])
    n0 = np.tanh(gx0[2 * H :] + r0 * gh0[2 * H :])
    h1 = (1.0 - z0) * n0                                    # h after step 0

    # --- length-sort, stratify across cores ---
    perm = np.argsort(lengths, kind="stable")
    lsort = lengths[perm]
    # pair p (on every core) covers global sorted ranks [p*8192, (p+1)*8192)
    pair_steps = [int(lsort[min((p + 1) * CH2 * NCORES, B) - 1]) - 1
                  for p in range(NP)]

    # --- shared weight tensors ---
    WXh = np.zeros((KX, 3 * H), f8)
    WXh[:V] = W_c.T
    WXh[V, H : 2 * H] = FREEZE
    wxd = WXh.astype(np_dt)
    whd = np.ascontiguousarray(W_hh.T).astype(np_dt)
    w1d = np.ascontiguousarray(W1.T).astype(np_dt)
    w2d = np.ascontiguousarray(W2.T).astype(np_dt)
    brzd = np.ascontiguousarray((b_c[: 2 * H] + b_hh[: 2 * H]).astype(np.float32).reshape(2 * H, 1))
    bhbd = np.ascontiguousarray(b_hh[2 * H :].astype(f8).reshape(1, H)).astype(np_dt)
    bcbd = np.ascontiguousarray(b_c[2 * H :].astype(f8).reshape(1, H)).astype(np_dt)
    b1d = np.ascontiguousarray(np.asarray(b1, np.float32).reshape(FC // 128, 128).T)
    b2f = (np.asarray(b2, f8) - W2.astype(f8).sum(axis=1)).astype(np.float32)
    b2p = np.zeros(OTILES * 128, np.float32)
    b2p[:OUT] = b2f
    b2d = np.ascontiguousarray(b2p.reshape(OTILES, 128).T)
    h1d = np.ascontiguousarray(h1.astype(np.float32).reshape(H, 1))

    # --- per-core inputs ---
    trange = np.arange(T - 1)
    in_maps = []
    core_idx = []
    for c in range(NCORES):
        ic = perm[c::NCORES]
        core_idx.append(ic)
        mc = message[ic][:, : T - 1, :]                     # [BC, 29, 21]
        Xc = np.empty((T - 1, KX, BC), dtype=np_dt)
        Xc[:, :V, :] = mc.transpose(1, 2, 0).astype(np_dt)
        Xc[:, V, :] = (lengths[ic][None, :] <= trange[:, None] + 1).astype(np_dt)
        in_maps.append({
            "X": Xc, "WX": wxd, "WH": whd, "W1T": w1d, "W2T": w2d,
            "BRZ": brzd, "BHB": bhbd, "BCB": bcbd, "B1": b1d, "B2": b2d,
            "H1": h1d,
        })

    nc = _build_nc(pair_steps, dt)
    res = run_bass_kernel_spmd(nc, in_maps, core_ids=list(range(NCORES)), trace=TRACE)
    LAST_RESULT = res

    out = np.empty((B, OUT), np.float32)
    for c in range(NCORES):
        out[core_idx[c]] = res.results[c]["OT"].T
    return out


# revision 28
# speedup vs baseline: 1.0791x; 1.0791x over previous
"""Trainium2 Bass kernel for nn_Decoder (GRU decoder over padded sequences).

Computation (per sample):
  emb = message[:, :T-1] @ W_emb.T + b_emb            (folded into W_c on host)
  xs  = [init_emb, emb]                                (step 0 folded into h1 const)
  GRU over T steps, gather h at lengths-1              (freeze trick: z := 1 past len)
  out = sigmoid(elu(h @ W1.T + b1) @ W2.T + b2)

Sharding: batch data-parallel over 8 cores, host-side length-sort (stratified
round-robin across cores) so each chunk only runs to its max length.

Device layout: features on partitions, batch on the free dim. The GRU state h
(and the weights it multiplies) live at partition base 64 so that the z-gate
slice of the sigmoid output tile (partitions 64..127) is base-aligned with the
h-update operands — every elementwise op's SBUF operands share a start
partition, which the ISA requires. Chunks are processed in pairs (1024
samples) with the h-update ops fused at FD=1024, and pairs are interleaved
PIPE at a time so the serial per-step dependency chain overlaps.
"""

import sys

sys.path.insert(0, "/opt/trn_rl_repo")

import numpy as np
import ml_dtypes

import concourse.bacc as bacc
import concourse.mybir as mybir
import concourse.tile as tile
from concourse.bass_utils import run_bass_kernel_spmd

B, T, V, E, H, FC, OUT = 65536, 30, 21, 32, 64, 256, 784
NCORES = 8
BC = B // NCORES          # 8192 samples per core
CH = 512                  # matmul free dim (one PSUM bank)
CH2 = 2 * CH              # pair width
NP = BC // CH2            # 8 pairs per core
FREEZE = 40.0             # z-gate preactivation offset for finished samples
KX = V + 1                # 21 msg rows + 1 freeze-flag row
OTILES = (OUT + 127) // 128  # 7 output row tiles
PIPE = 8                  # pairs in flight (all interleaved)
XB = 2                    # GRU steps per streamed X block

USE_BF16 = True
GP_HP = False             # final h-update add on GpSimd (else VectorE)
HN_COPY = True            # stage hn PSUM->SBUF via ScalarE so t1 runs at 2x
TRACE = False             # set by test harness for profiling
LAST_RESULT = None        # BassKernelResults stash for the harness

_f32 = mybir.dt.float32
_bf16 = mybir.dt.bfloat16


def _sigmoid(x):
    return 1.0 / (1.0 + np.exp(-x))


def _build_nc(pair_steps, dt):
    """Build the SPMD kernel. pair_steps[p] = GRU steps (beyond the constant
    step 0) for pair p — identical on every core."""
    AF = mybir.ActivationFunctionType
    OP = mybir.AluOpType
    nc = bacc.Bacc("TRN2", target_bir_lowering=False, debug=False)

    X = nc.dram_tensor("X", [T - 1, KX, BC], dt, kind="ExternalInput")
    WX = nc.dram_tensor("WX", [KX, 3 * H], dt, kind="ExternalInput")
    WH = nc.dram_tensor("WH", [H, 3 * H], dt, kind="ExternalInput")
    W1T = nc.dram_tensor("W1T", [H, FC], dt, kind="ExternalInput")
    W2T = nc.dram_tensor("W2T", [FC, OUT], dt, kind="ExternalInput")
    BRZ = nc.dram_tensor("BRZ", [2 * H, 1], _f32, kind="ExternalInput")
    B1 = nc.dram_tensor("B1", [128, FC // 128], _f32, kind="ExternalInput")
    B2 = nc.dram_tensor("B2", [128, OTILES], _f32, kind="ExternalInput")
    H1 = nc.dram_tensor("H1", [H, 1], _f32, kind="ExternalInput")
    BHB = nc.dram_tensor("BHB", [1, H], dt, kind="ExternalInput")
    BCB = nc.dram_tensor("BCB", [1, H], dt, kind="ExternalInput")
    OT = nc.dram_tensor("OT", [OUT, BC], _f32, kind="ExternalOutput")

    ew = nc.gpsimd if GP_HP else nc.vector

    with tile.TileContext(nc) as tc:
        with (
            tc.tile_pool(name="weights", bufs=1) as wp,
            tc.tile_pool(name="xin", bufs=9) as xp,
            tc.tile_pool(name="hstate", bufs=NP) as hp,
            tc.tile_pool(name="hfinal", bufs=1) as hf,
            tc.tile_pool(name="gates", bufs=3) as gp,
            tc.tile_pool(name="head", bufs=3) as fp,
            tc.tile_pool(name="frhs", bufs=1) as frp,
            tc.tile_pool(name="outs", bufs=3) as op_,
            tc.tile_pool(name="psA", bufs=2, space="PSUM") as psA,
            tc.tile_pool(name="psB", bufs=2, space="PSUM") as psB,
        ):
            # --- load weights/biases once; h-side weights live at base 64 ---
            wx = wp.tile([128, 3 * H], dt)
            nc.vector.memset(wx[:], 0.0)
            nc.sync.dma_start(out=wx[0:KX, :], in_=WX[:])
            # ones-row bias folding: x row KX is 1.0, h row 0 is 1.0; the
            # matching weight rows carry b_cn / b_hn into the n-path PSUM.
            nc.sync.dma_start(out=wx[32:33, 2 * H :], in_=BCB[:])
            whF = wp.tile([128, 3 * H], dt)
            nc.vector.memset(whF[0:H, :], 0.0)
            nc.sync.dma_start(out=whF[H:128, :], in_=WH[:])
            nc.sync.dma_start(out=whF[0:1, 2 * H :], in_=BHB[:])
            w1F = wp.tile([128, FC], dt)
            nc.vector.memset(w1F[0:H, :], 0.0)
            nc.sync.dma_start(out=w1F[H:128, :], in_=W1T[:])
            w2a = wp.tile([128, OUT], dt)
            nc.sync.dma_start(out=w2a[:], in_=W2T[0:128, :])
            w2b = wp.tile([128, OUT], dt)
            nc.sync.dma_start(out=w2b[:], in_=W2T[128:256, :])
            brz = wp.tile([2 * H, 1], _f32)
            nc.sync.dma_start(out=brz[:], in_=BRZ[:])
            b1s = wp.tile([128, FC // 128], _f32)
            nc.sync.dma_start(out=b1s[:], in_=B1[:])
            b2s = wp.tile([128, OTILES], _f32)
            nc.sync.dma_start(out=b2s[:], in_=B2[:])
            h1F = wp.tile([128, 1], _f32)
            nc.sync.dma_start(out=h1F[H:128, :], in_=H1[:])

            hfin = []

            # PE warm-up: ~5us of dense junk matmuls so HAM un-throttles the
            # clock gate (4/8 -> 8/8) before the GRU loops start.
            warm = psA.tile([128, CH2], _f32, tag="ps")
            for _ in range(30):
                nc.tensor.matmul(
                    warm[:, 0 : 3 * H], wx[:, 0:128], wx[:], start=True, stop=True
                )

            for _xi in range(9):
                xz = xp.tile([128, XB, CH2], dt, tag="xt")
                nc.vector.memset(xz[:], 0.0)
                nc.vector.memset(xz[32:33, :, :], 1.0)

            def start_pair(p):
                """Allocate tiles + init h for pair p; returns state dict."""
                ha = hp.tile([128, CH2], dt, tag="hpi")
                hb = hp.tile([128, CH2], dt, tag="hpo")
                hfp = hf.tile([128, CH2], dt, tag=f"hf{p}")
                hfin.append(hfp)
                nc.vector.memset(ha[:], 0.0)
                nc.vector.memset(ha[0:1, :], 1.0)
                nc.vector.memset(hb[0:H, :], 0.0)
                nc.vector.memset(hb[0:1, :], 1.0)
                nc.vector.memset(hfp[0:H, :], 0.0)
                nc.vector.memset(hfp[0:1, :], 1.0)
                nc.vector.tensor_scalar_add(ha[H:128, :], ha[H:128, :], h1F[H:128, :])
                if pair_steps[p] == 0:
                    nc.vector.memset(hfp[H:128, :], 0.0)
                    nc.vector.tensor_scalar_add(
                        hfp[H:128, :], hfp[H:128, :], h1F[H:128, :]
                    )
                return {"xt": None, "cur": ha, "nxt": hb, "hf": hfp, "p": p}

            def emit_mms(bite, s):
                """Gate matmuls for up to 2 pairs at step s, grouped by weight
                matrix so consecutive MMs share LDWEIGHTS and pipeline."""
                for st in bite:
                    nsteps = pair_steps[st["p"]]
                    if (s - 1) % XB == 0:
                        nb = min(XB, nsteps - (s - 1))
                        xtn = xp.tile([128, XB, CH2], dt, tag="xt")
                        st["xt"] = xtn
                        ps = slice(st["p"] * CH2, (st["p"] + 1) * CH2)
                        nc.sync.dma_start(
                            out=st["xt"][0:KX, 0:nb, :],
                            in_=X[s - 1 : s - 1 + nb, :, ps].rearrange("t k b -> k t b"),
                        )
                    # 2-bank PSUM tiles: both chunks side by side -> FD=1024 ops.
                    # pn layout: hn on partitions 0..63, xn on partitions 64..127.
                    prz2 = psA.tile([128, CH2], _f32, tag="ps")
                    pn2 = psB.tile([128, CH2], _f32, tag="pn")
                    st["prz2"], st["pn2"] = prz2, pn2

                def each(fn):
                    for st in bite:
                        j = (s - 1) % XB
                        for g in (0, 1):
                            gs = slice(g * CH, (g + 1) * CH)
                            fn(st, st["xt"][:, j, gs], st["cur"][:, gs], gs)

                each(lambda st, xs_, cur_g, gs: nc.tensor.matmul(
                    st["prz2"][:, gs], wx[:, 0:128], xs_, start=True, stop=False))
                each(lambda st, xs_, cur_g, gs: nc.tensor.matmul(
                    st["prz2"][:, gs], whF[:, 0:128], cur_g, start=False, stop=True))
                each(lambda st, xs_, cur_g, gs: nc.tensor.matmul(
                    st["pn2"][0:H, gs], whF[:, 128:192], cur_g, start=True, stop=True))
                each(lambda st, xs_, cur_g, gs: nc.tensor.matmul(
                    st["pn2"][H:128, gs], wx[:, 128:192], xs_, start=True, stop=True))

            def emit_step(st, s):
                """Elementwise GRU step for both chunks of a pair."""
                cur = st["cur"]
                nsteps = pair_steps[st["p"]]
                prz2, pn2 = st["prz2"], st["pn2"]
                rzF = gp.tile([128, CH2], dt, tag="rz")
                nc.scalar.activation(rzF[:], prz2[:], AF.Sigmoid, bias=brz[:])
                # biases b_hn / b_cn already folded into pn2 via ones-rows.
                # Stage hn to SBUF via ScalarE so t1 runs as bf16 2x TT.
                t1 = gp.tile([H, CH2], dt, tag="t1")
                if HN_COPY:
                    hns = gp.tile([H, CH2], dt, tag="hns")
                    nc.scalar.copy(hns[:], pn2[0:H, :])
                    nc.vector.tensor_mul(t1[:], hns[:], rzF[0:H, :])
                else:
                    nc.vector.tensor_mul(t1[:], pn2[0:H, :], rzF[0:H, :])
                t2F = gp.tile([H, CH2], dt, tag="t2")
                nc.vector.tensor_add(t2F[:], pn2[H:128, :], t1[:])
                # n = tanh(t2);  h' = n + z*(h - n)
                ntF = gp.tile([128, CH2], dt, tag="nt")
                nc.scalar.activation(ntF[H:128, :], t2F[:], AF.Tanh)
                uF = gp.tile([128, CH2], dt, tag="u")
                nc.vector.tensor_sub(uF[H:128, :], cur[H:128, :], ntF[H:128, :])
                vF = gp.tile([128, CH2], dt, tag="v")
                nc.vector.tensor_mul(vF[H:128, :], rzF[H:128, :], uF[H:128, :])
                dst = st["hf"] if s == nsteps else st["nxt"]
                ew.tensor_add(dst[H:128, :], ntF[H:128, :], vF[H:128, :])
                st["cur"], st["nxt"] = dst, st["cur"]

            # --- GRU loops: all pairs interleaved so short pairs' tails
            # overlap long pairs' matmul phases ---
            grp = [start_pair(p) for p in range(NP)]
            maxs = max(pair_steps[st["p"]] for st in grp)
            for s in range(1, maxs + 1):
                active = [st for st in grp if s <= pair_steps[st["p"]]]
                for i in range(0, len(active), 2):
                    bite = active[i : i + 2]
                    emit_mms(bite, s)
                    for st in bite:
                        emit_step(st, s)

            # --- MLP heads, two barrier-separated phases so ACT runs all its
            # Exp calls together (one table swap) then all Sigmoids ---
            tc.no_sync_barrier()
            f2rhs = {}
            for p in range(NP):
                for g in (0, 1):
                    gs = slice(g * CH, (g + 1) * CH)
                    cur_g = hfin[p][:, gs]
                    for j in range(FC // 128):
                        pfF = psA.tile([128, CH2], _f32, tag="ps")
                        pf = pfF[:, 0:CH]
                        nc.tensor.matmul(
                            pf, w1F[:, j * 128 : (j + 1) * 128], cur_g,
                            start=True, stop=True,
                        )
                        # elu(x+b1) + 1 == (x+b1 - m) + exp(m),  m = min(x+b1, 0)
                        m = fp.tile([128, CH], dt, tag="m")
                        nc.vector.tensor_scalar(
                            m[:], pf, b1s[:, j : j + 1], 0.0, op0=OP.add, op1=OP.min
                        )
                        e = fp.tile([128, CH], dt, tag="e")
                        nc.scalar.activation(e[:], m[:], AF.Exp)
                        pp = fp.tile([128, CH], dt, tag="pp")
                        nc.vector.scalar_tensor_tensor(
                            pp[:], pf, b1s[:, j : j + 1], m[:],
                            op0=OP.add, op1=OP.subtract,
                        )
                        fr = frp.tile([128, CH], dt, tag=f"fr{p}{g}{j}")
                        nc.vector.tensor_add(fr[:], pp[:], e[:])
                        f2rhs[(p, g, j)] = fr

            tc.no_sync_barrier()
            for p in range(NP):
                for g in (0, 1):
                    for mt in range(OTILES):
                        mw = min(128, OUT - mt * 128)
                        poF = psB.tile([128, CH2], _f32, tag="pn")
                        po = poF[0:mw, 0:CH]
                        nc.tensor.matmul(
                            po, w2a[:, mt * 128 : mt * 128 + mw],
                            f2rhs[(p, g, 0)][:], start=True, stop=False,
                        )
                        nc.tensor.matmul(
                            po, w2b[:, mt * 128 : mt * 128 + mw],
                            f2rhs[(p, g, 1)][:], start=False, stop=True,
                        )
                        ot = op_.tile([mw, CH], _f32, tag="ot")
                        nc.scalar.activation(
                            ot[:], po, AF.Sigmoid, bias=b2s[0:mw, mt : mt + 1]
                        )
                        nc.sync.dma_start(
                            out=OT[mt * 128 : mt * 128 + mw, p * CH2 + g * CH :
                                   p * CH2 + (g + 1) * CH],
                            in_=ot[:],
                        )

    nc.compile()
    return nc


def kernel(message, lengths, init_emb, W_emb, b_emb, W_ih, W_hh, b_ih, b_hh,
           W1, b1, W2, b2):
    global LAST_RESULT
    message = np.asarray(message, dtype=np.float32)
    lengths = np.asarray(lengths).astype(np.int64)
    f8 = np.float64
    np_dt = ml_dtypes.bfloat16 if USE_BF16 else np.float32
    dt = _bf16 if USE_BF16 else _f32

    # --- fold embedding into input weights;  step 0 is a constant ---
    W_c = W_ih.astype(f8) @ W_emb.astype(f8)                # [3H, V]
    b_c = W_ih.astype(f8) @ b_emb.astype(f8) + b_ih         # [3H]
    gx0 = W_ih.astype(f8) @ init_emb.astype(f8) + b_ih
    gh0 = b_hh.astype(f8)
    r0 = _sigmoid(gx0[:H] + gh0[:H])
    z0 = _sigmoid(gx0[H : 2 * H] + gh0[H : 2 * H])
    n0 = np.tanh(gx0[2 * H :] + r0 * gh0[2 * H :])
    h1 = (1.0 - z0) * n0                                    # h after step 0

    # --- length-sort, stratify across cores ---
    perm = np.argsort(lengths, kind="stable")
    lsort = lengths[perm]
    # pair p (on every core) covers global sorted ranks [p*8192, (p+1)*8192)
    pair_steps = [int(lsort[min((p + 1) * CH2 * NCORES, B) - 1]) - 1
                  for p in range(NP)]

    # --- shared weight tensors ---
    WXh = np.zeros((KX, 3 * H), f8)
    WXh[:V] = W_c.T
    WXh[V, H : 2 * H] = FREEZE
    wxd = WXh.astype(np_dt)
    whd = np.ascontiguousarray(W_hh.T).astype(np_dt)
    w1d = np.ascontiguousarray(W1.T).astype(np_dt)
    w2d = np.ascontiguousarray(W2.T).astype(np_dt)
    brzd = np.ascontiguousarray((b_c[: 2 * H] + b_hh[: 2 * H]).astype(np.float32).reshape(2 * H, 1))
    bhbd = np.ascontiguousarray(b_hh[2 * H :].astype(f8).reshape(1, H)).astype(np_dt)
    bcbd = np.ascontiguousarray(b_c[2 * H :].astype(f8).reshape(1, H)).astype(np_dt)
    b1d = np.ascontiguousarray(np.asarray(b1, np.float32).reshape(FC // 128, 128).T)
    b2f = (np.asarray(b2, f8) - W2.astype(f8).sum(axis=1)).astype(np.float32)
    b2p = np.zeros(OTILES * 128, np.float32)
    b2p[:OUT] = b2f
    b2d = np.ascontiguousarray(b2p.reshape(OTILES, 128).T)
    h1d = np.ascontiguousarray(h1.astype(np.float32).reshape(H, 1))

    # --- per-core inputs ---
    trange = np.arange(T - 1)
    in_maps = []
    core_idx = []
    for c in range(NCORES):
        ic = perm[c::NCORES]
        core_idx.append(ic)
        mc = message[ic][:, : T - 1, :]                     # [BC, 29, 21]
        Xc = np.empty((T - 1, KX, BC), dtype=np_dt)
        Xc[:, :V, :] = mc.transpose(1, 2, 0).astype(np_dt)
        Xc[:, V, :] = (lengths[ic][None, :] <= trange[:, None] + 1).astype(np_dt)
        in_maps.append({
            "X": Xc, "WX": wxd, "WH": whd, "W1T": w1d, "W2T": w2d,
            "BRZ": brzd, "BHB": bhbd, "BCB": bcbd, "B1": b1d, "B2": b2d,
            "H1": h1d,
        })

    nc = _build_nc(pair_steps, dt)
    res = run_bass_kernel_spmd(nc, in_maps, core_ids=list(range(NCORES)), trace=TRACE)
    LAST_RESULT = res

    out = np.empty((B, OUT), np.float32)
    for c in range(NCORES):
        out[core_idx[c]] = res.results[c]["OT"].T
    return out
